# revision 47
# baseline (speedup 1.0000x reference)
"""Trainium2 Bass kernel for llama-style GQA attention block (v2).

Problem (hardcoded): x[1,2048,2048] f32, 32 q heads / 8 kv heads, head_dim 64,
RoPE (interleaved pairs), causal mask, out proj. 8-core tensor parallel across
heads: each core owns 4 q heads + 1 kv head, computes its slice end-to-end
including its wo row-block partial product; host sums the 8 partials.

All matmuls run as float32r (fp32 data, fast PE mode). Feature-on-partition
layout throughout:
  QT = wq^T x^T       (via lhsT=wq, rhs=xT)
  ST = K Q^T          (via lhsT=Krot, rhs=Qrot) -> softmax along partitions
  OT = [V|1]^T PT     (via lhsT=Vext, rhs=PT)   -> row 64 = softmax denominator
  out = OT^T wo       (via lhsT=OT, rhs=wo)

v2 layout: wq columns are permuted host-side so each PSUM projection tile is
per-head contiguous: tile A rows = [h0 evens(32), h0 odds(32), h1 evens,
h1 odds], tile B = heads 2,3. After RoPE each head's rotated Q occupies 64
contiguous partitions, so each score tile is ONE matmul contracting 64
partitions (vs 2x 32-contraction in v1). K (single kv head) is rotated into
[Ke;Ko] on 64 partitions and replicated once to partitions 64:128 so both
heads of a pair align with their Q partitions.
"""

import numpy as np

import concourse.bass as bass
import concourse.bacc as bacc
import concourse.mybir as mybir
from concourse.tile import TileContext
from concourse.bass_utils import run_bass_kernel_spmd

DIM = 2048
SEQ = 2048
N_HEADS = 32
N_KV = 8
HD = 64
NCORES = 8
HPC = N_HEADS // NCORES      # 4 q heads per core
SC = 512                     # seq chunk (matmul free dim)
NSC = SEQ // SC              # 4
KTILE = 128
NKT = SEQ // KTILE           # 16
NDCH = DIM // 128            # 16 contraction chunks for projections
F32 = mybir.dt.float32
F32R = mybir.dt.float32r
BF16 = mybir.dt.bfloat16
NEG = -1.0e30

_CACHE = {}


def _build_nc(reps=1, reload_weights=True):
    nc = bacc.Bacc("TRN2", debug=False, num_devices=NCORES)
    xT_p = nc.declare_dram_parameter("xT", [DIM, SEQ], BF16, isOutput=False)
    wq_p = nc.declare_dram_parameter("wq", [DIM, HPC * HD], BF16, isOutput=False)
    wkv_p = nc.declare_dram_parameter("wkv", [DIM, 2 * HD], BF16, isOutput=False)
    wo_p = nc.declare_dram_parameter("wo", [HPC * HD, DIM], F32R, isOutput=False)
    cs_p = nc.declare_dram_parameter("cs", [256, SEQ], F32, isOutput=False)
    pat_p = nc.declare_dram_parameter("pat", [KTILE, KTILE], BF16, isOutput=False)
    id_p = nc.declare_dram_parameter("ident", [128, 128], F32, isOutput=False)
    out_p = nc.declare_dram_parameter("out", [SEQ, DIM], F32, isOutput=True)

    xT_r = xT_p.rearrange("(k p) s -> p k s", p=128)
    wq_r = wq_p.rearrange("(k p) m -> p k m", p=128)
    wkv_r = wkv_p.rearrange("(k p) m -> p k m", p=128)
    EXP = mybir.ActivationFunctionType.Exp

    with TileContext(nc) as tc:
        with (
            tc.tile_pool(name="res", bufs=1) as res,
            tc.tile_pool(name="sb", bufs=2) as sb,
            tc.tile_pool(name="psum", bufs=1, space="PSUM") as psum,
        ):
            ps_ctr = [0]

            def ps_tile(idx, shape=(128, SC)):
                ps_ctr[0] += 1
                return psum.tile(list(shape), F32, tag=f"p{idx}", name=f"ps{ps_ctr[0]}")

            # ---- resident allocs ----
            wq_t = res.tile([128, NDCH, HPC * HD], BF16, tag="wq_t")
            wkv_t = res.tile([128, NDCH, 2 * HD], BF16, tag="wkv_t")
            wo0_t = res.tile([128, DIM], F32R, tag="wo0_t")
            wo1_t = res.tile([128, DIM], F32R, tag="wo1_t")
            cos4 = res.tile([128, SEQ], F32, tag="cos4")
            sin4 = res.tile([128, SEQ], F32, tag="sin4")
            pat_t = res.tile([128, 128], BF16, tag="pat_t")
            patf_t = res.tile([128, 128], F32, tag="patf_t")
            ident = res.tile([128, 128], F32, tag="ident")
            id128r = res.tile([128, 128], BF16, tag="id128r")

            # rotated K, both pair-slots: [Ke;Ko] at 0:64 and again at 64:128
            Krep = res.tile([128, SEQ], F32R, tag="Krep")
            # transposed V for all k-tiles; cols HD:HD+64 all-ones so the PV
            # matmul emits the softmax denominator replicated on 64 partitions
            vext = res.tile([128, NKT, 2 * HD], F32R, tag="vext")
            ones_blk = res.tile([128, HD], F32, tag="ones_blk")
            nc.vector.memset(ones_blk[:], 1.0)
            for kt in range(NKT):
                nc.vector.tensor_copy(vext[:, kt, HD : 2 * HD], ones_blk[:])

            for _rep in range(reps):
                for sc in range(NSC):
                    slc = slice(sc * SC, (sc + 1) * SC)
                    # ---- xt load: 4 batched DMAs for this chunk ----
                    xt = sb.tile([128, NDCH, SC], BF16, tag="xt", bufs=2)
                    for g in range(0, NDCH, 4):
                        nc.gpsimd.dma_start(
                            out=xt[:, g : g + 4, :], in_=xT_r[:, g : g + 4, slc]
                        )
                    # ---- proj(sc): QKV projections (banks p0, p1, p2) ----
                    qa_ps = ps_tile(0)
                    qb_ps = ps_tile(1)
                    kv_ps = ps_tile(2)
                    def load_weights(k):
                        nc.sync.dma_start(out=wq_t[:, k, :], in_=wq_r[:, k, :])
                        nc.sync.dma_start(out=wkv_t[:, k, :], in_=wkv_r[:, k, :])
                        if k == 1:
                            nc.sync.dma_start(out=cos4[:], in_=cs_p[0:128, :])
                            nc.sync.dma_start(out=sin4[:], in_=cs_p[128:256, :])
                        if k == 2:
                            nc.sync.dma_start(out=pat_t[:], in_=pat_p[:, :])
                            nc.sync.dma_start(out=ident[:], in_=id_p[:, :])
                            # casting DMA (gpsimd): f32 identity -> bf16
                            nc.gpsimd.dma_start(out=id128r[:], in_=id_p[:, :])
                            nc.vector.tensor_copy(patf_t[:], pat_t[:])
                        if k == 8:
                            nc.sync.dma_start(out=wo0_t[:], in_=wo_p[0:128, :])
                        if k == 12:
                            nc.sync.dma_start(out=wo1_t[:], in_=wo_p[128:256, :])

                    for k in range(NDCH):
                        if sc == 0 and _rep == 0:
                            load_weights(k)
                        st, sp = (k == 0), (k == NDCH - 1)
                        xk = xt[:, k, :]
                        nc.tensor.matmul(qa_ps[:], wq_t[:, k, 0:128], xk, start=st, stop=sp)
                        nc.tensor.matmul(qb_ps[:], wq_t[:, k, 128:256], xk, start=st, stop=sp)
                        nc.tensor.matmul(kv_ps[:], wkv_t[:, k, :], xk, start=st, stop=sp)
                        if sc == NSC - 1 and reload_weights and _rep < reps - 1:
                            # prefetch next rep's (identical) weights right after
                            # this k-slice's last use, hiding the reload under
                            # the rep tail (attention + outproj of this chunk)
                            load_weights(k)

                    # ---- rope K + replicate; V passthrough (first: gates scores) ----
                    kc = sb.tile([64, SC], F32, tag="kc", bufs=2)
                    ks = sb.tile([64, SC], F32, tag="ks", bufs=2)
                    kw = sb.tile([64, SC], F32, tag="kw", bufs=2)
                    nc.vector.tensor_mul(kc[:], kv_ps[0:64, :], cos4[0:64, slc])
                    nc.vector.tensor_mul(ks[:], kv_ps[0:64, :], sin4[0:64, slc])
                    nc.vector.tensor_copy(kw[0:32, :], ks[32:64, :])
                    nc.vector.tensor_copy(kw[32:64, :], ks[0:32, :])
                    nc.vector.tensor_add(Krep[0:64, slc], kc[:], kw[:])
                    nc.vector.tensor_copy(Krep[64:128, slc], Krep[0:64, slc])

                    # ---- rope Q: per-head-contiguous tiles ----
                    # sin4 carries alternating signs [+s;-s] per 32-row block, so
                    # after a 32-block swap of ts the rotation is a single add:
                    #   rot = A*cos + swap32(A*sin_eff)
                    QP = []
                    for ti, qps in ((0, qa_ps), (1, qb_ps)):
                        qp = sb.tile([128, SC], F32R, tag=f"QP{ti}", bufs=2)
                        tc_ = sb.tile([128, SC], F32, tag="tcq", bufs=2)
                        ts_ = sb.tile([128, SC], F32, tag="tsq", bufs=2)
                        tw_ = sb.tile([128, SC], F32, tag="twq", bufs=2)
                        nc.vector.tensor_mul(tc_[:], qps[:], cos4[:, slc])
                        nc.vector.tensor_mul(ts_[:], qps[:], sin4[:, slc])
                        for b in range(4):
                            r = 32 * b
                            src = r + 32 if b % 2 == 0 else r - 32
                            nc.vector.tensor_copy(tw_[r : r + 32, :], ts_[src : src + 32, :])
                        nc.vector.tensor_add(qp[:], tc_[:], tw_[:])
                        QP.append(qp)

                    # ---- V transpose, 4 tiles in one PSUM bank ----
                    vt_sb = sb.tile([HD, SC], F32, tag="vt_sb", bufs=2)
                    nc.vector.tensor_copy(vt_sb[:], kv_ps[HD:128, :])
                    vt_ps = ps_tile(2, (128, 4, HD))
                    for j in range(4):
                        nc.tensor.transpose(
                            vt_ps[:, j, :], vt_sb[:, j * 128 : (j + 1) * 128],
                            ident[0:HD, 0:HD],
                        )
                    nc.vector.tensor_copy(
                        vext[:, 4 * sc : 4 * sc + 4, 0:HD], vt_ps[:, :, :]
                    )

                    # ---- attention(sc): head pairs, st pair-tile spans banks p3+p4,
                    #      o accumulators banks p5/p6 ----
                    nkt_h = 4 * sc + 4
                    OTn = []
                    for hp in range(2):
                        heads = (2 * hp, 2 * hp + 1)
                        o_ps = {h: ps_tile(5 + i, (128, SC)) for i, h in enumerate(heads)}
                        for kt in range(nkt_h):
                            ksl = slice(kt * 128, (kt + 1) * 128)
                            j = kt - 4 * sc
                            qo = 128 * j if j > 0 else 0
                            nv = SC - qo
                            st_ps = {h: ps_tile(3 + i) for i, h in enumerate(heads)}
                            for i, h in enumerate(heads):
                                part = slice(64 * (h % 2), 64 * (h % 2) + 64)
                                qpt = QP[h // 2]
                                nc.tensor.matmul(
                                    st_ps[h][:, 0:nv],
                                    Krep[part, ksl],
                                    qpt[part, qo:SC],
                                    start=True, stop=True,
                                )
                                if j >= 0:
                                    # causal triangle mask on the diagonal block
                                    nc.vector.tensor_add(
                                        st_ps[h][:, 0:128], st_ps[h][:, 0:128], patf_t[:]
                                    )
                            for h in heads:
                                ptile = sb.tile([128, SC], F32R, tag="pt", bufs=8)
                                nc.scalar.activation(
                                    ptile[:, 0:nv], st_ps[h][:, 0:nv], EXP, scale=0.125
                                )
                                nc.tensor.matmul(
                                    o_ps[h][:, qo : qo + nv],
                                    vext[:, kt, :],
                                    ptile[:, 0:nv],
                                    start=(kt == 0), stop=(kt == nkt_h - 1),
                                )
                        otn = sb.tile([128, SC], F32R, tag=f"OTn{hp}", bufs=2)
                        recip = sb.tile([128, SC], F32R, tag="recip", bufs=2)
                        for h in heads:
                            hh = h % 2
                            rsl = slice(64 * hh, 64 * hh + 64)
                            with nc.allow_low_precision(reason="f32r is fp32-width"):
                                nc.vector.reciprocal(recip[rsl, :], o_ps[h][HD:128, :])
                            nc.vector.tensor_mul(
                                otn[rsl, :], o_ps[h][0:HD, :], recip[rsl, :]
                            )
                        OTn.append(otn)

                    # ---- outproj(sc): rows of this chunk (bank p7) ----
                    for sti in range(4):
                        st_g = 4 * sc + sti
                        ssl = slice(sti * 128, (sti + 1) * 128)
                        osl = slice(st_g * 128, (st_g + 1) * 128)
                        ot = sb.tile([128, DIM], F32, tag="ot", bufs=2)
                        for dch in range(NSC):
                            dsl = slice(dch * SC, (dch + 1) * SC)
                            op_ps = ps_tile(7)
                            nc.tensor.matmul(op_ps[:], OTn[0][:, ssl], wo0_t[:, dsl], start=True, stop=False)
                            nc.tensor.matmul(op_ps[:], OTn[1][:, ssl], wo1_t[:, dsl], start=False, stop=True)
                            if dch % 2 == 0:
                                nc.vector.tensor_copy(ot[:, dsl], op_ps[:])
                            else:
                                nc.scalar.activation(
                                    ot[:, dsl], op_ps[:],
                                    mybir.ActivationFunctionType.Copy,
                                )
                        nc.gpsimd.dma_start(out=out_p[osl, :], in_=ot[:])

    nc.compile()
    return nc


def _host_prep(x, freqs_cos, freqs_sin):
    """Shared (core-independent) host-side tensors."""
    xT = np.ascontiguousarray(np.asarray(x, np.float32)[0].T)          # [DIM, SEQ]
    cosT = np.ascontiguousarray(np.asarray(freqs_cos, np.float32).T)   # [32, SEQ]
    sinT = np.ascontiguousarray(np.asarray(freqs_sin, np.float32).T)
    # sin rows alternate sign per 32-block: [+s; -s; +s; -s] so that
    # rot = A*cos + swap32(A*sin_eff) gives (e*c - o*s, o*c + e*s)
    sin_eff = np.concatenate([sinT, -sinT, sinT, -sinT], 0)            # [128, SEQ]
    cs = np.concatenate([np.tile(cosT, (4, 1)), sin_eff], 0)           # [256, SEQ]
    kk = np.arange(KTILE)[:, None]
    qq = np.arange(KTILE)[None, :]
    pat = np.where(kk <= qq, 0.0, NEG).astype(np.float32)              # [128, 128]
    return xT, cs, pat


def _perm_q():
    """wq columns -> per-head-contiguous [h evens(32), h odds(32)] blocks."""
    p = []
    for h in range(HPC):
        p += [h * HD + 2 * i for i in range(HD // 2)]
        p += [h * HD + 2 * i + 1 for i in range(HD // 2)]
    return p


def _perm_k():
    """wk columns (single head) -> [even dims (32), odd dims (32)]."""
    return [2 * i for i in range(HD // 2)] + [2 * i + 1 for i in range(HD // 2)]


def _is_causal(mask):
    m = np.asarray(mask)
    if m.shape != (SEQ, SEQ):
        return False
    tril = np.tril(np.ones((SEQ, SEQ), bool))
    return bool(np.all(m[tril] == 0.0) and np.all(np.isneginf(m[~tril])))


def _numpy_fallback(x, freqs_cos, freqs_sin, mask, wq, wk, wv, wo):
    x = np.asarray(x, np.float64)
    b, s, _ = x.shape
    xq = (x @ wq).reshape(b, s, N_HEADS, HD)
    xk = (x @ wk).reshape(b, s, N_KV, HD)
    xv = (x @ wv).reshape(b, s, N_KV, HD)

    def rope(t):
        t2 = t.reshape(*t.shape[:-1], HD // 2, 2)
        te, to = t2[..., 0], t2[..., 1]
        c = np.asarray(freqs_cos, np.float64)[None, :, None, :]
        sn = np.asarray(freqs_sin, np.float64)[None, :, None, :]
        oe = te * c - to * sn
        oo = te * sn + to * c
        return np.stack([oe, oo], -1).reshape(t.shape)

    xq, xk = rope(xq), rope(xk)
    xk = np.repeat(xk, N_HEADS // N_KV, axis=2)
    xv = np.repeat(xv, N_HEADS // N_KV, axis=2)
    sc_ = np.einsum("bqhd,bkhd->bhqk", xq, xk) / np.sqrt(HD)
    sc_ = sc_ + np.asarray(mask, np.float64)[None, None]
    m = sc_.max(-1, keepdims=True)
    p = np.exp(sc_ - m)
    p = p / p.sum(-1, keepdims=True)
    out = np.einsum("bhqk,bkhd->bqhd", p, xv).reshape(b, s, N_HEADS * HD)
    return (out @ wo).astype(np.float32)


def _make_in_maps(x, freqs_cos, freqs_sin, wq, wk, wv, wo):
    import ml_dtypes
    bf16 = ml_dtypes.bfloat16
    xT, cs, pat = _host_prep(x, freqs_cos, freqs_sin)
    xT = np.ascontiguousarray(xT.astype(bf16))
    pat = np.ascontiguousarray(pat.astype(bf16))
    wq = np.asarray(wq, np.float32)
    wk = np.asarray(wk, np.float32)
    wv = np.asarray(wv, np.float32)
    wo = np.asarray(wo, np.float32)
    permq = _perm_q()
    permk = _perm_k()
    in_maps = []
    for c in range(NCORES):
        wq_c = np.ascontiguousarray(wq[:, c * 256 : (c + 1) * 256][:, permq].astype(bf16))
        wk_c = wk[:, c * HD : (c + 1) * HD][:, permk]
        wv_c = wv[:, c * HD : (c + 1) * HD]
        wkv_c = np.ascontiguousarray(np.concatenate([wk_c, wv_c], 1).astype(bf16))
        wo_c = np.ascontiguousarray(wo[c * 256 : (c + 1) * 256, :])
        im = {"xT": xT, "wq": wq_c, "wkv": wkv_c, "wo": wo_c, "cs": cs, "pat": pat,
              "ident": np.eye(128, dtype=np.float32)}
        expect = {"xT": (DIM, SEQ), "wq": (DIM, HPC * HD), "wkv": (DIM, 2 * HD),
                  "wo": (HPC * HD, DIM), "cs": (256, SEQ), "pat": (KTILE, KTILE),
                  "ident": (128, 128)}
        for k_, v_ in im.items():
            assert v_.shape == expect[k_], (k_, v_.shape, expect[k_])
        in_maps.append(im)
    return in_maps


def get_nc(reps=1, reload_weights=True):
    key = f"nc{reps}_{reload_weights}"
    if key not in _CACHE:
        _CACHE[key] = _build_nc(reps, reload_weights)
    return _CACHE[key]


def kernel(x, freqs_cos, freqs_sin, mask, wq, wk, wv, wo):
    if not _is_causal(mask):
        return _numpy_fallback(x, freqs_cos, freqs_sin, mask, wq, wk, wv, wo)
    nc = get_nc()
    in_maps = _make_in_maps(x, freqs_cos, freqs_sin, wq, wk, wv, wo)
    res = run_bass_kernel_spmd(nc, in_maps, list(range(NCORES))).results
    acc = res[0]["out"].astype(np.float64)
    for c in range(1, NCORES):
        acc += res[c]["out"]
    return acc.astype(np.float32)[None]
